# revision 16
# baseline (speedup 1.0000x reference)
"""Bidirectional cross-attention layer on 8 Trainium2 NeuronCores.

Strategy: pure data-parallel over batch (B=64 -> 8 per core), no collectives.
Per core, activations flow in "transposed" layout [feature | rows] so every
projection is a PE matmul with contraction on the partition dim; fp32r
(TF32-like) matmuls run at full PE rate.  rows are b-major: r = b*N + n.

Layer (per core, batch slice of 8):
  t  = tokens + CrossAttn(LN(tokens), image) ; t += FFN(LN(t))
  im = image  + CrossAttn(LN(image), tokens) ; im += FFN(LN(im))
(the image branch attends to the ORIGINAL tokens, so branches are independent)
"""

import numpy as np

import concourse.bass as bass
import concourse.mybir as mybir
import concourse.tile as tile
from concourse import bacc
from concourse.bass_utils import run_bass_kernel_spmd
from concourse.masks import make_identity

P = 128
NT, NI, D, H, DH, HID = 256, 576, 1024, 16, 64, 4096
NCORES, BL = 8, 8
RT, RI = NT * BL, NI * BL        # 2048, 4608 rows per core
KC = D // P                      # 8 feature chunks
SCALE = DH ** -0.5
F32 = mybir.dt.float32
F32R = mybir.dt.float32r
AF = mybir.ActivationFunctionType
ALU = mybir.AluOpType

WNAMES = [
    "tn_g", "tn_b", "t2i_wq", "t2i_wk", "t2i_wv", "t2i_wo", "t2i_bo",
    "in_g", "in_b", "i2t_wq", "i2t_wk", "i2t_wv", "i2t_wo", "i2t_bo",
    "tfn_g", "tfn_b", "tf_w1", "tf_b1", "tf_w2", "tf_b2",
    "ifn_g", "ifn_b", "if_w1", "if_b1", "if_w2", "if_b2",
]
WSHAPES = {
    "tn_g": [D], "tn_b": [D], "in_g": [D], "in_b": [D],
    "tfn_g": [D], "tfn_b": [D], "ifn_g": [D], "ifn_b": [D],
    "t2i_wq": [D, D], "t2i_wk": [D, D], "t2i_wv": [D, D], "t2i_wo": [D, D],
    "i2t_wq": [D, D], "i2t_wk": [D, D], "i2t_wv": [D, D], "i2t_wo": [D, D],
    "t2i_bo": [D], "i2t_bo": [D],
    "tf_w1": [D, HID], "tf_b1": [HID], "tf_w2": [HID, D], "tf_b2": [D],
    "if_w1": [D, HID], "if_b1": [HID], "if_w2": [HID, D], "if_b2": [D],
}

_CACHE = {}


def dram_bcast(ap_1d, nparts):
    """AP reading a 1-D DRAM vector replicated across nparts partitions."""
    return bass.AP(tensor=ap_1d.tensor, offset=ap_1d.offset,
                   ap=[[0, nparts]] + list(ap_1d.ap))


def build():
    nc = bacc.Bacc()
    toks = nc.declare_dram_parameter("tokens", [NT, BL, D], F32, isOutput=False)
    imgf = nc.declare_dram_parameter("image_features", [NI, BL, D], F32,
                                     isOutput=False)
    W = {n: nc.declare_dram_parameter(n, WSHAPES[n], F32, isOutput=False)
         for n in WNAMES}
    t_out = nc.declare_dram_parameter("t_out", [NT, BL, D], F32, isOutput=True)
    im_out = nc.declare_dram_parameter("im_out", [NI, BL, D], F32,
                                       isOutput=True)

    tok_bnd = toks.rearrange("n b d -> b n d")          # r = b*NT + n
    img_bnd = imgf.rearrange("n b d -> b n d")
    tout_bnd = t_out.rearrange("n b d -> b n d")
    iout_bnd = im_out.rearrange("n b d -> b n d")

    def rows_spans(r0, cnt, npb):
        spans = []
        off = 0
        while cnt > 0:
            b, n0 = divmod(r0, npb)
            take = min(cnt, npb - n0)
            spans.append((off, b, n0, take))
            r0 += take
            cnt -= take
            off += take
        return spans

    def dma_rows(tile_ap, src3d, r0, cnt, npb, store=False):
        for off, b, n0, take in rows_spans(r0, cnt, npb):
            if store:
                nc.sync.dma_start(src3d[b, n0:n0 + take, :],
                                  tile_ap[off:off + take, :])
            else:
                nc.sync.dma_start(tile_ap[off:off + take, :],
                                  src3d[b, n0:n0 + take, :])

    def w_kxn(name):
        """weight [K, N] dram -> AP [p, kc, n] for SBUF [128, K/128, N]."""
        return W[name].rearrange("(kc p) n -> p kc n", p=P).bitcast(F32R)

    def vec_col(name):
        """1-D [D or HID] -> AP [p, chunks] (column layout, f32)."""
        return W[name].rearrange("(c p) -> p c", p=P)

    with tile.TileContext(nc) as tc:
        # ------------------------------------------------- persistent state
        dram_cm = tc.tile_pool(name="dram", bufs=1, space="DRAM")
        dram = dram_cm.__enter__()
        TtT = dram.tile([P, KC, RT], F32R)      # raw tokens, transposed
        TiT = dram.tile([P, KC, RI], F32R)      # raw image, transposed
        LtT = dram.tile([P, KC, RT], F32R)      # LN(tokens), transposed
        LiT = dram.tile([P, KC, RI], F32R)      # LN(image), transposed
        qtT = dram.tile([P, KC, RT], F32R)
        kiT = dram.tile([P, KC, RI], F32R)
        qiT = dram.tile([P, KC, RI], F32R)
        ktT = dram.tile([P, KC, RT], F32R)
        vpad_i = dram.tile([RI, H * (DH + 1)], F32R)
        vpad_t = dram.tile([RT, H * (DH + 1)], F32R)
        attnT_t = dram.tile([P, KC, RT], F32R)
        attnT_i = dram.tile([P, KC, RI], F32R)
        tmid = dram.tile([RT, D], F32)
        imid = dram.tile([RI, D], F32)
        xhT_t = dram.tile([P, KC, RT], F32R)
        xhT_i = dram.tile([P, KC, RI], F32R)
        scr_t = dram.tile([BL, H, NT], F32)
        scr_i = dram.tile([BL, H, NI], F32)
        scr2_t = dram.tile([BL, H, NT], F32)
        scr2_i = dram.tile([BL, H, NI], F32)
        y0_t = dram.tile([RT, D], F32)
        y0_i = dram.tile([RI, D], F32)

        singles_cm = tc.tile_pool(name="singles", bufs=1)
        singles = singles_cm.__enter__()
        ident32 = singles.tile([P, P], F32)
        make_identity(nc, ident32)
        identr = singles.tile([P, P], F32R)
        nc.vector.tensor_copy(identr[:], ident32[:])
        eps_t = singles.tile([P, 1], F32)
        nc.vector.memset(eps_t[:], 1e-5)
        ones16 = singles.tile([P, H], F32)
        nc.vector.memset(ones16[:], 1.0)
        # per-feature vectors in column layout [128, KC]
        gcols = {}
        for n in ("tn_g", "tn_b", "in_g", "in_b", "tfn_g", "tfn_b",
                  "ifn_g", "ifn_b"):
            gcols[n] = singles.tile([P, KC], F32, name=f"col_{n}")
            nc.sync.dma_start(gcols[n][:], vec_col(n))
        b1col = {}
        for n in ("tf_b1", "if_b1"):
            b1col[n] = singles.tile([P, HID // P], F32, name=f"col_{n}")
            nc.sync.dma_start(b1col[n][:], vec_col(n))

        # ---------------------------------------------------------- helpers
        def stats_ln(pool, x):
            """x: [128, D] f32 sbuf -> (mean, rstd) [128,1] f32."""
            st = pool.tile([P, 2, 6], F32, tag="st")
            xv = x.rearrange("p (s f) -> p s f", s=2)
            nc.vector.bn_stats(st[:, 0], xv[:, 0])
            nc.vector.bn_stats(st[:, 1], xv[:, 1])
            mv = pool.tile([P, 2], F32, tag="mv")
            nc.vector.bn_aggr(mv[:], st[:])
            rs = pool.tile([P, 1], F32, tag="rs")
            nc.scalar.activation(rs[:], mv[:, 1:2], AF.Sqrt, bias=eps_t[:])
            nc.vector.reciprocal(rs[:], rs[:])
            return mv, rs

        def prep_stage(name, src3d, npb, ROWS, gn, bn, rawT, lnT,
                       inner=None):
            """rows -> rawT + LN'd-transposed (with gamma/beta applied)."""
            with (
                tc.tile_pool(name=f"prep_{name}", bufs=3) as pool,
                tc.tile_pool(name=f"prepb_{name}", bufs=2) as bpool,
                tc.tile_pool(name=f"prep_ps_{name}", bufs=2,
                             space="PSUM") as psp,
                tc.tile_pool(name=f"prep_ps2_{name}", bufs=2,
                             space="PSUM") as psp2,
            ):
                for blk in range(ROWS // 512):
                    r0 = blk * 512
                    raw_blk = bpool.tile([P, KC, 512], F32R, tag="rawblk")
                    ln_blk = bpool.tile([P, KC, 512], F32R, tag="lnblk")
                    for t4 in range(4):
                        x = pool.tile([P, D], F32, tag="x")
                        dma_rows(x, src3d, r0 + t4 * P, P, npb)
                        mv, rs = stats_ln(pool, x)
                        xc = pool.tile([P, D], F32R, tag="xc")
                        nc.vector.tensor_scalar(
                            xc[:], x[:], scalar1=mv[:, 0:1], scalar2=rs[:],
                            op0=ALU.subtract, op1=ALU.mult)
                        for c in range(KC):
                            pr = psp.tile([P, P], F32, tag="tr32")
                            nc.tensor.transpose(
                                pr[:], x[:, c * P:(c + 1) * P], ident32)
                            nc.scalar.copy(
                                out=raw_blk[:, c, t4 * P:(t4 + 1) * P],
                                in_=pr[:])
                            pc = psp2.tile([P, P], F32R, tag="trr")
                            nc.tensor.transpose(
                                pc[:], xc[:, c * P:(c + 1) * P], identr)
                            nc.vector.tensor_copy(
                                ln_blk[:, c, t4 * P:(t4 + 1) * P], pc[:])
                    for c in range(KC):
                        nc.scalar.activation(
                            ln_blk[:, c, :], ln_blk[:, c, :], AF.Identity,
                            bias=gcols[bn][:, c:c + 1],
                            scale=gcols[gn][:, c:c + 1])
                    nc.gpsimd.dma_start(rawT[:, :, r0:r0 + 512], raw_blk[:])
                    nc.gpsimd.dma_start(lnT[:, :, r0:r0 + 512], ln_blk[:])
                if inner is not None:
                    inner()

        def proj_T(name, srcT, ROWS, wname, dstT):
            """dstT = (src @ w) transposed-out, streaming 512-row blocks."""
            with (
                tc.tile_pool(name=f"pt_{name}", bufs=2) as pool,
                tc.tile_pool(name=f"pt_w_{name}", bufs=1) as wpool,
                tc.tile_pool(name=f"pt_ps_{name}", bufs=4, space="PSUM") as ps,
            ):
                w_sb = wpool.tile([P, KC, D], F32R)
                nc.sync.dma_start(w_sb[:], w_kxn(wname))
                for blk in range(ROWS // 512):
                    r0 = blk * 512
                    src_blk = pool.tile([P, KC, 512], F32R, tag="src")
                    nc.sync.dma_start(src_blk[:], srcT[:, :, r0:r0 + 512])
                    o_blk = pool.tile([P, KC, 512], F32R, tag="o")
                    for m in range(KC):
                        acc = ps.tile([P, 512], F32, tag="acc")
                        for kc in range(KC):
                            nc.tensor.matmul(
                                acc[:], w_sb[:, kc, m * P:(m + 1) * P],
                                src_blk[:, kc, :],
                                start=(kc == 0), stop=(kc == KC - 1))
                        nc.vector.tensor_copy(o_blk[:, m, :], acc[:])
                    nc.gpsimd.dma_start(dstT[:, :, r0:r0 + 512], o_blk[:])

        def proj_vpad(name, srcT, ROWS, wname, vpad):
            """v = src @ wv row-major, padded per head with a ones column."""
            with (
                tc.tile_pool(name=f"pv_{name}", bufs=2) as pool,
                tc.tile_pool(name=f"pv_w_{name}", bufs=1) as wpool,
                tc.tile_pool(name=f"pv_ps_{name}", bufs=4, space="PSUM") as ps,
            ):
                w_sb = wpool.tile([P, KC, D], F32R)
                nc.sync.dma_start(w_sb[:], w_kxn(wname))
                for blk in range(ROWS // 512):
                    r0 = blk * 512
                    src_blk = pool.tile([P, KC, 512], F32R, tag="src")
                    nc.sync.dma_start(src_blk[:], srcT[:, :, r0:r0 + 512])
                    for rc in range(4):
                        vp = pool.tile([P, H, DH + 1], F32R, tag="vp")
                        nc.vector.tensor_copy(vp[:, :, DH:DH + 1],
                                              ones16[:, :, None])
                        for nn in range(2):
                            acc = ps.tile([P, 512], F32, tag="acc")
                            for kc in range(KC):
                                nc.tensor.matmul(
                                    acc[:],
                                    src_blk[:, kc, rc * P:(rc + 1) * P],
                                    w_sb[:, kc, nn * 512:(nn + 1) * 512],
                                    start=(kc == 0), stop=(kc == KC - 1))
                            nc.vector.tensor_copy(
                                vp[:, nn * 8:(nn + 1) * 8, :DH],
                                acc.rearrange("p (h d) -> p h d", h=8))
                        nc.gpsimd.dma_start(
                            vpad[r0 + rc * P:r0 + (rc + 1) * P, :],
                            vp.rearrange("p h d -> p (h d)"))

        def attn(name, qT, nq, kT, nk, vpad, scr, scr2, dstT):
            """Cross attention: per b, per head; q rows nq, kv rows nk."""
            nkc = (nk + P - 1) // P
            pieces = [(0, nq)] if nq <= 512 else [(0, nq // 2),
                                                  (nq // 2, nq // 2)]
            with (
                tc.tile_pool(name=f"at_{name}", bufs=2) as pool,
                tc.tile_pool(name=f"at_t1_{name}", bufs=1) as tailp,
                tc.tile_pool(name=f"at_sm_{name}", bufs=2) as smp,
                tc.tile_pool(name=f"at_ex_{name}", bufs=4) as exp_pool,
                tc.tile_pool(name=f"at_ps_{name}", bufs=2, space="PSUM") as ps,
                tc.tile_pool(name=f"at_po_{name}", bufs=4,
                             space="PSUM") as pso,
            ):
                for b in range(BL):
                    kT_b = pool.tile([P, KC, nk], F32R, tag="kT")
                    nc.sync.dma_start(kT_b[:], kT[:, :, b * nk:(b + 1) * nk])
                    qT_b = pool.tile([P, KC, nq], F32R, tag="qT")
                    nc.sync.dma_start(qT_b[:], qT[:, :, b * nq:(b + 1) * nq])
                    vps = []
                    for c in range(nkc):
                        ckn = min(P, nk - c * P)
                        vt = pool.tile([P, H * (DH + 1)], F32R, tag=f"vp{c}")
                        nc.sync.dma_start(
                            vt[:ckn], vpad[b * nk + c * P:b * nk + c * P + ckn])
                        vps.append((vt, ckn))
                    oU = pool.tile([P, KC, nq], F32, tag="oU")
                    dall = tailp.tile([1, H * nq], F32, tag="dall")
                    for hp in range(H // 2):
                        h0, h1 = 2 * hp, 2 * hp + 1
                        for (qoff, qn) in pieces:
                            acc_o0 = pso.tile([DH + 1, 512], F32, tag="acco")
                            acc_o1 = pso.tile([DH + 1, 512], F32, tag="acco")
                            for c in range(nkc):
                                vt, ckn = vps[c]
                                # both heads' scores in one 2-bank psum tile;
                                # one Exp over the pair halves the ACT count
                                acc_s = ps.tile([P, 2, 512], F32, tag="accs")
                                nc.tensor.matmul(
                                    acc_s[:ckn, 0, :qn],
                                    kT_b[0:DH, hp, c * P:c * P + ckn],
                                    qT_b[0:DH, hp, qoff:qoff + qn],
                                    start=True, stop=True)
                                nc.tensor.matmul(
                                    acc_s[:ckn, 1, :qn],
                                    kT_b[DH:P, hp, c * P:c * P + ckn],
                                    qT_b[DH:P, hp, qoff:qoff + qn],
                                    start=True, stop=True)
                                et = exp_pool.tile([P, 2, 512], F32R,
                                                   tag="et")
                                nc.scalar.activation(
                                    et[:ckn, :, :qn], acc_s[:ckn, :, :qn],
                                    AF.Exp, scale=SCALE)
                                nc.tensor.matmul(
                                    acc_o0[:, :qn],
                                    vt[:ckn,
                                       h0 * (DH + 1):(h0 + 1) * (DH + 1)],
                                    et[:ckn, 0, :qn],
                                    start=(c == 0), stop=(c == nkc - 1))
                                nc.tensor.matmul(
                                    acc_o1[:, :qn],
                                    vt[:ckn,
                                       h1 * (DH + 1):(h1 + 1) * (DH + 1)],
                                    et[:ckn, 1, :qn],
                                    start=(c == 0), stop=(c == nkc - 1))
                            for h, acc_o, po in ((h0, acc_o0, 0),
                                                 (h1, acc_o1, DH)):
                                nc.vector.tensor_copy(
                                    dall[:, h * nq + qoff:
                                         h * nq + qoff + qn],
                                    acc_o[DH:DH + 1, :qn])
                                nc.vector.tensor_copy(
                                    oU[po:po + DH, hp, qoff:qoff + qn],
                                    acc_o[:DH, :qn])
                    # denominator: bounce through DRAM to reshape [1, H*nq]
                    # -> [H, nq], reciprocal, bounce back broadcast [128,KC,nq]
                    scr_b = scr[b]
                    nc.gpsimd.dma_start(
                        scr_b.rearrange("h q -> (h q)")[None, :], dall[:])
                    den16 = smp.tile([H, nq], F32, tag="den16")
                    nc.gpsimd.dma_start(den16[:], scr_b)
                    nc.vector.reciprocal(den16[:], den16[:])
                    scr2_b = scr2[b]
                    nc.gpsimd.dma_start(scr2_b, den16[:])
                    bc = tailp.tile([P, KC, nq], F32, tag="bc")
                    for h2 in range(2):
                        sl = scr2_b[h2, :]
                        nc.gpsimd.dma_start(
                            bc[h2 * 64:(h2 + 1) * 64, :, :],
                            bass.AP(tensor=sl.tensor, offset=sl.offset,
                                    ap=[[0, 64], [2 * nq, KC], [1, nq]]))
                    obn = tailp.tile([P, KC, nq], F32R, tag="obn")
                    nc.vector.tensor_mul(obn[:], oU[:], bc[:])
                    nc.gpsimd.dma_start(dstT[:, :, b * nq:(b + 1) * nq],
                                        obn[:])

        def mid_stage(name, attnT, ROWS, woname, boname, res3d, npb, gn, bn,
                      mid, xhT):
            """o-proj + bias + residual -> mid; then LN -> xhat transposed."""
            with (
                tc.tile_pool(name=f"md_{name}", bufs=3) as pool,
                tc.tile_pool(name=f"md_b_{name}", bufs=2) as bpool,
                tc.tile_pool(name=f"md_w_{name}", bufs=1) as wpool,
                tc.tile_pool(name=f"md_ps_{name}", bufs=4, space="PSUM") as ps,
                tc.tile_pool(name=f"md_ps2_{name}", bufs=2,
                             space="PSUM") as psp2,
            ):
                w_sb = wpool.tile([P, KC, D], F32R)
                nc.sync.dma_start(w_sb[:], w_kxn(woname))
                bo_bc = wpool.tile([P, D], F32)
                nc.sync.dma_start(bo_bc[:], dram_bcast(W[boname][:], P))
                for blk in range(ROWS // 512):
                    r0 = blk * 512
                    a_blk = bpool.tile([P, KC, 512], F32R, tag="ablk")
                    nc.sync.dma_start(a_blk[:], attnT[:, :, r0:r0 + 512])
                    xh_blk = bpool.tile([P, KC, 512], F32R, tag="xhblk")
                    for rc in range(4):
                        rr = r0 + rc * P
                        t1 = pool.tile([P, D], F32, tag="t1")
                        res = pool.tile([P, D], F32, tag="res")
                        dma_rows(res, res3d, rr, P, npb)
                        for nn in range(2):
                            acc = ps.tile([P, 512], F32, tag="acc")
                            for kc in range(KC):
                                nc.tensor.matmul(
                                    acc[:], a_blk[:, kc, rc * P:(rc + 1) * P],
                                    w_sb[:, kc, nn * 512:(nn + 1) * 512],
                                    start=(kc == 0), stop=(kc == KC - 1))
                            nc.vector.tensor_add(
                                t1[:, nn * 512:(nn + 1) * 512], acc[:],
                                res[:, nn * 512:(nn + 1) * 512])
                        nc.vector.tensor_add(t1[:], t1[:], bo_bc[:])
                        nc.gpsimd.dma_start(mid[rr:rr + P, :], t1[:])
                        mv, rs = stats_ln(pool, t1)
                        xc = pool.tile([P, D], F32R, tag="xc")
                        nc.vector.tensor_scalar(
                            xc[:], t1[:], scalar1=mv[:, 0:1], scalar2=rs[:],
                            op0=ALU.subtract, op1=ALU.mult)
                        for c in range(KC):
                            pc = psp2.tile([P, P], F32R, tag="trr")
                            nc.tensor.transpose(
                                pc[:], xc[:, c * P:(c + 1) * P], identr)
                            nc.scalar.copy(
                                out=xh_blk[:, c, rc * P:(rc + 1) * P],
                                in_=pc[:])
                    for c in range(KC):
                        nc.scalar.activation(
                            xh_blk[:, c, :], xh_blk[:, c, :], AF.Identity,
                            bias=gcols[bn][:, c:c + 1],
                            scale=gcols[gn][:, c:c + 1])
                    nc.gpsimd.dma_start(xhT[:, :, r0:r0 + 512], xh_blk[:])

        def ffn_stage(name, xhT, ROWS, w1name, b1name, w2name, b2name,
                      mid, y0, out3d, npb):
            """h = gelu(xh @ w1 + b1); y = h @ w2 + b2 + mid.

            HID processed in quarters of 1024 with a ping-pong weight pool
            (bufs=2): quarter q+1 weights DMA during quarter q compute."""
            QKC = 8                    # 1024/128 k-chunks per quarter
            with (
                tc.tile_pool(name=f"ff_{name}", bufs=2) as pool,
                tc.tile_pool(name=f"ff_b_{name}", bufs=1) as bpool,
                tc.tile_pool(name=f"ff_w_{name}", bufs=2) as wpool,
                tc.tile_pool(name=f"ff_ps_{name}", bufs=4,
                             space="PSUM") as ps,
                tc.tile_pool(name=f"ff_ps2_{name}", bufs=4,
                             space="PSUM") as ps2,
            ):
                b2_bc = bpool.tile([P, D], F32)
                nc.sync.dma_start(b2_bc[:], dram_bcast(W[b2name][:], P))
                for qt in range(4):
                    w1_sb = wpool.tile([P, KC, 1024], F32R, tag="w1")
                    nc.sync.dma_start(
                        w1_sb[:],
                        w_kxn(w1name)[:, :, qt * 1024:(qt + 1) * 1024])
                    w2_sb = wpool.tile([P, QKC, D], F32R, tag="w2")
                    nc.sync.dma_start(
                        w2_sb[:], w_kxn(w2name)[:, qt * QKC:(qt + 1) * QKC, :])
                    for blk in range(ROWS // 256):
                        r0 = blk * 256
                        xh_blk = pool.tile([P, KC, 256], F32R, tag="xh")
                        nc.sync.dma_start(xh_blk[:], xhT[:, :, r0:r0 + 256])
                        hT_blk = pool.tile([P, QKC, 256], F32R, tag="hT")
                        for m in range(QKC):
                            acc = ps.tile([P, 256], F32, tag="acc")
                            for kc in range(KC):
                                nc.tensor.matmul(
                                    acc[:], w1_sb[:, kc, m * P:(m + 1) * P],
                                    xh_blk[:, kc, :],
                                    start=(kc == 0), stop=(kc == KC - 1))
                            nc.scalar.activation(
                                hT_blk[:, m, :], acc[:], AF.Gelu,
                                bias=b1col[b1name][:, qt * QKC + m:
                                                   qt * QKC + m + 1])
                        for rc in range(2):
                            rr = r0 + rc * P
                            yt = pool.tile([P, D], F32, tag="yt")
                            for nn in range(2):
                                acc2 = ps2.tile([P, 512], F32, tag="acc2")
                                for kc in range(QKC):
                                    nc.tensor.matmul(
                                        acc2[:],
                                        hT_blk[:, kc, rc * P:(rc + 1) * P],
                                        w2_sb[:, kc, nn * 512:(nn + 1) * 512],
                                        start=(kc == 0), stop=(kc == QKC - 1))
                                if qt == 0:
                                    nc.vector.tensor_add(
                                        yt[:, nn * 512:(nn + 1) * 512],
                                        acc2[:],
                                        b2_bc[:, nn * 512:(nn + 1) * 512])
                                else:
                                    y0t = pool.tile([P, 512], F32, tag="y0t")
                                    nc.sync.dma_start(
                                        y0t[:],
                                        y0[rr:rr + P,
                                           nn * 512:(nn + 1) * 512])
                                    nc.vector.tensor_add(
                                        yt[:, nn * 512:(nn + 1) * 512],
                                        acc2[:], y0t[:])
                            if qt < 3:
                                nc.gpsimd.dma_start(y0[rr:rr + P, :], yt[:])
                            else:
                                mt = pool.tile([P, D], F32, tag="mt")
                                nc.sync.dma_start(mt[:], mid[rr:rr + P, :])
                                nc.vector.tensor_add(yt[:], yt[:], mt[:])
                                dma_rows(yt, out3d, rr, P, npb,
                                         store=True)

        # interleave the two independent branches: every stage's producer
        # is >=2 stages back, so adjacent stages pipeline on the engines
        with nc.named_scope("prep_t"):
            prep_stage("t", tok_bnd, NT, RT, "tn_g", "tn_b", TtT, LtT)
        def _t_projs():
            # PE-dense projections depending only on prep_t outputs; run
            # inside prep_i's pool scope so the stack allocator places them
            # above prep_i's live pools -> no reuse dep -> true overlap of
            # prep_i's DVE work with PE matmuls.
            proj_T("qt", LtT, RT, "t2i_wq", qtT)
            proj_T("kt", TtT, RT, "i2t_wk", ktT)
            proj_vpad("vt", TtT, RT, "i2t_wv", vpad_t)

        with nc.named_scope("prep_i"):
            prep_stage("i", img_bnd, NI, RI, "in_g", "in_b", TiT, LiT,
                       inner=_t_projs)
        with nc.named_scope("proj_ki"):
            proj_T("ki", TiT, RI, "t2i_wk", kiT)
        with nc.named_scope("proj_vi"):
            proj_vpad("vi", TiT, RI, "t2i_wv", vpad_i)
        with nc.named_scope("attn_t2i"):
            attn("t2i", qtT, NT, kiT, NI, vpad_i, scr_t, scr2_t, attnT_t)
        with nc.named_scope("proj_qi"):
            proj_T("qi", LiT, RI, "i2t_wq", qiT)
        with nc.named_scope("attn_i2t"):
            attn("i2t", qiT, NI, ktT, NT, vpad_t, scr_i, scr2_i, attnT_i)
        with nc.named_scope("mid_t"):
            mid_stage("t", attnT_t, RT, "t2i_wo", "t2i_bo", tok_bnd, NT,
                      "tfn_g", "tfn_b", tmid, xhT_t)
        with nc.named_scope("mid_i"):
            mid_stage("i", attnT_i, RI, "i2t_wo", "i2t_bo", img_bnd, NI,
                      "ifn_g", "ifn_b", imid, xhT_i)
        with nc.named_scope("ffn_t"):
            ffn_stage("t", xhT_t, RT, "tf_w1", "tf_b1", "tf_w2", "tf_b2",
                      tmid, y0_t, tout_bnd, NT)
        with nc.named_scope("ffn_i"):
            ffn_stage("i", xhT_i, RI, "if_w1", "if_b1", "if_w2", "if_b2",
                      imid, y0_i, iout_bnd, NI)

        singles_cm.__exit__(None, None, None)
        dram_cm.__exit__(None, None, None)

    nc.compile()
    return nc


def kernel(**inputs):
    if "nc" not in _CACHE:
        _CACHE["nc"] = build()
    nc = _CACHE["nc"]
    in_maps = []
    for core in range(NCORES):
        b0 = core * BL
        m = {"tokens": np.ascontiguousarray(inputs["tokens"][:, b0:b0 + BL]),
             "image_features": np.ascontiguousarray(
                 inputs["image_features"][:, b0:b0 + BL])}
        for n in WNAMES:
            m[n] = np.asarray(inputs[n], dtype=np.float32)
        in_maps.append(m)
    res = run_bass_kernel_spmd(nc, in_maps, list(range(NCORES)))
    t = np.concatenate([r["t_out"] for r in res.results], axis=1)
    im = np.concatenate([r["im_out"] for r in res.results], axis=1)
    return (t, im)


if __name__ == "__main__":
    rng = np.random.default_rng(0)
    ins = {
        "tokens": rng.standard_normal((NT, BL * NCORES, D)).astype(np.float32),
        "image_features": rng.standard_normal((NI, BL * NCORES, D)).astype(
            np.float32),
    }
    for n in WNAMES:
        sh = WSHAPES[n]
        if n.endswith("_g"):
            ins[n] = np.ones(sh, np.float32)
        elif n.endswith(("_b", "_bo", "b1", "b2")):
            ins[n] = np.zeros(sh, np.float32)
        else:
            ins[n] = (rng.standard_normal(sh) * 0.02).astype(np.float32)
    t, im = kernel(**ins)
    print("t", t.shape, "im", im.shape)


# revision 18
# speedup vs baseline: 2.0392x; 2.0392x over previous
"""Bidirectional cross-attention layer on 8 Trainium2 NeuronCores.

Strategy: pure data-parallel over batch (B=64 -> 8 per core), no collectives.
Per core, activations flow in "transposed" layout [feature | rows] so every
projection is a PE matmul with contraction on the partition dim; fp32r
(TF32-like) matmuls run at full PE rate.  rows are b-major: r = b*N + n.

Layer (per core, batch slice of 8):
  t  = tokens + CrossAttn(LN(tokens), image) ; t += FFN(LN(t))
  im = image  + CrossAttn(LN(image), tokens) ; im += FFN(LN(im))
(the image branch attends to the ORIGINAL tokens, so branches are independent)
"""

import numpy as np

import concourse.bass as bass
import concourse.mybir as mybir
import concourse.tile as tile
from concourse import bacc
from concourse.bass_utils import run_bass_kernel_spmd
from concourse.masks import make_identity

P = 128
NT, NI, D, H, DH, HID = 256, 576, 1024, 16, 64, 4096
NCORES, BL = 8, 8
RT, RI = NT * BL, NI * BL        # 2048, 4608 rows per core
KC = D // P                      # 8 feature chunks
SCALE = DH ** -0.5
F32 = mybir.dt.float32
F32R = mybir.dt.float32r
AF = mybir.ActivationFunctionType
ALU = mybir.AluOpType

WNAMES = [
    "tn_g", "tn_b", "t2i_wq", "t2i_wk", "t2i_wv", "t2i_wo", "t2i_bo",
    "in_g", "in_b", "i2t_wq", "i2t_wk", "i2t_wv", "i2t_wo", "i2t_bo",
    "tfn_g", "tfn_b", "tf_w1", "tf_b1", "tf_w2", "tf_b2",
    "ifn_g", "ifn_b", "if_w1", "if_b1", "if_w2", "if_b2",
]
WSHAPES = {
    "tn_g": [D], "tn_b": [D], "in_g": [D], "in_b": [D],
    "tfn_g": [D], "tfn_b": [D], "ifn_g": [D], "ifn_b": [D],
    "t2i_wq": [D, D], "t2i_wk": [D, D], "t2i_wv": [D, D], "t2i_wo": [D, D],
    "i2t_wq": [D, D], "i2t_wk": [D, D], "i2t_wv": [D, D], "i2t_wo": [D, D],
    "t2i_bo": [D], "i2t_bo": [D],
    "tf_w1": [D, HID], "tf_b1": [HID], "tf_w2": [HID, D], "tf_b2": [D],
    "if_w1": [D, HID], "if_b1": [HID], "if_w2": [HID, D], "if_b2": [D],
}

_CACHE = {}


def dram_bcast(ap_1d, nparts):
    """AP reading a 1-D DRAM vector replicated across nparts partitions."""
    return bass.AP(tensor=ap_1d.tensor, offset=ap_1d.offset,
                   ap=[[0, nparts]] + list(ap_1d.ap))


def build():
    nc = bacc.Bacc()
    toks = nc.declare_dram_parameter("tokens", [NT, BL, D], F32, isOutput=False)
    imgf = nc.declare_dram_parameter("image_features", [NI, BL, D], F32,
                                     isOutput=False)
    W = {n: nc.declare_dram_parameter(n, WSHAPES[n], F32, isOutput=False)
         for n in WNAMES}
    t_out = nc.declare_dram_parameter("t_out", [NT, BL, D], F32, isOutput=True)
    im_out = nc.declare_dram_parameter("im_out", [NI, BL, D], F32,
                                       isOutput=True)

    tok_bnd = toks.rearrange("n b d -> b n d")          # r = b*NT + n
    img_bnd = imgf.rearrange("n b d -> b n d")
    tout_bnd = t_out.rearrange("n b d -> b n d")
    iout_bnd = im_out.rearrange("n b d -> b n d")

    def rows_spans(r0, cnt, npb):
        spans = []
        off = 0
        while cnt > 0:
            b, n0 = divmod(r0, npb)
            take = min(cnt, npb - n0)
            spans.append((off, b, n0, take))
            r0 += take
            cnt -= take
            off += take
        return spans

    def dma_rows(tile_ap, src3d, r0, cnt, npb, store=False):
        for off, b, n0, take in rows_spans(r0, cnt, npb):
            if store:
                nc.sync.dma_start(src3d[b, n0:n0 + take, :],
                                  tile_ap[off:off + take, :])
            else:
                nc.sync.dma_start(tile_ap[off:off + take, :],
                                  src3d[b, n0:n0 + take, :])

    def w_kxn(name):
        """weight [K, N] dram -> AP [p, kc, n] for SBUF [128, K/128, N]."""
        return W[name].rearrange("(kc p) n -> p kc n", p=P).bitcast(F32R)

    def vec_col(name):
        """1-D [D or HID] -> AP [p, chunks] (column layout, f32)."""
        return W[name].rearrange("(c p) -> p c", p=P)

    with tile.TileContext(nc) as tc:
        # ------------------------------------------------- persistent state
        dram_cm = tc.tile_pool(name="dram", bufs=1, space="DRAM")
        dram = dram_cm.__enter__()
        TtT = dram.tile([P, KC, RT], F32R)      # raw tokens, transposed
        TiT = dram.tile([P, KC, RI], F32R)      # raw image, transposed
        LtT = dram.tile([P, KC, RT], F32R)      # LN(tokens), transposed
        LiT = dram.tile([P, KC, RI], F32R)      # LN(image), transposed
        qtT = dram.tile([P, KC, RT], F32R)
        kiT = dram.tile([P, KC, RI], F32R)
        qiT = dram.tile([P, KC, RI], F32R)
        ktT = dram.tile([P, KC, RT], F32R)
        vpad_i = dram.tile([RI, H * (DH + 1)], F32R)
        vpad_t = dram.tile([RT, H * (DH + 1)], F32R)
        attnT_t = dram.tile([P, KC, RT], F32R)
        attnT_i = dram.tile([P, KC, RI], F32R)
        tmid = dram.tile([RT, D], F32)
        imid = dram.tile([RI, D], F32)
        xhT_t = dram.tile([P, KC, RT], F32R)
        xhT_i = dram.tile([P, KC, RI], F32R)
        scr_t = dram.tile([BL, H, NT], F32)
        scr_i = dram.tile([BL, H, NI], F32)
        scr2_t = dram.tile([BL, H, NT], F32)
        scr2_i = dram.tile([BL, H, NI], F32)
        y0_t = dram.tile([RT, D], F32)
        y0_i = dram.tile([RI, D], F32)

        singles_cm = tc.tile_pool(name="singles", bufs=1)
        singles = singles_cm.__enter__()
        ident32 = singles.tile([P, P], F32)
        make_identity(nc, ident32)
        identr = singles.tile([P, P], F32R)
        nc.vector.tensor_copy(identr[:], ident32[:])
        eps_t = singles.tile([P, 1], F32)
        nc.vector.memset(eps_t[:], 1e-5)
        ones16 = singles.tile([P, H], F32)
        nc.vector.memset(ones16[:], 1.0)
        # per-feature vectors in column layout [128, KC]
        gcols = {}
        for n in ("tn_g", "tn_b", "in_g", "in_b", "tfn_g", "tfn_b",
                  "ifn_g", "ifn_b"):
            gcols[n] = singles.tile([P, KC], F32, name=f"col_{n}")
            nc.sync.dma_start(gcols[n][:], vec_col(n))
        b1col = {}
        for n in ("tf_b1", "if_b1"):
            b1col[n] = singles.tile([P, HID // P], F32, name=f"col_{n}")
            nc.sync.dma_start(b1col[n][:], vec_col(n))

        # ---------------------------------------------------------- helpers
        def stats_ln(pool, x):
            """x: [128, D] f32 sbuf -> (mean, rstd) [128,1] f32."""
            st = pool.tile([P, 2, 6], F32, tag="st")
            xv = x.rearrange("p (s f) -> p s f", s=2)
            nc.vector.bn_stats(st[:, 0], xv[:, 0])
            nc.vector.bn_stats(st[:, 1], xv[:, 1])
            mv = pool.tile([P, 2], F32, tag="mv")
            nc.vector.bn_aggr(mv[:], st[:])
            rs = pool.tile([P, 1], F32, tag="rs")
            nc.scalar.activation(rs[:], mv[:, 1:2], AF.Sqrt, bias=eps_t[:])
            nc.vector.reciprocal(rs[:], rs[:])
            return mv, rs

        def prep_stage(name, src3d, npb, ROWS, gn, bn, rawT, lnT,
                       inner=None):
            """rows -> rawT + LN'd-transposed (with gamma/beta applied)."""
            with (
                tc.tile_pool(name=f"prep_{name}", bufs=4) as pool,
                tc.tile_pool(name=f"prepb_{name}", bufs=2) as bpool,
                tc.tile_pool(name=f"prep_ps_{name}", bufs=2,
                             space="PSUM") as psp,
                tc.tile_pool(name=f"prep_ps2_{name}", bufs=2,
                             space="PSUM") as psp2,
            ):
                for blk in range(ROWS // 512):
                    r0 = blk * 512
                    raw_blk = bpool.tile([P, KC, 512], F32R, tag="rawblk")
                    ln_blk = bpool.tile([P, KC, 512], F32R, tag="lnblk")
                    for t4 in range(4):
                        x = pool.tile([P, D], F32, tag="x")
                        dma_rows(x, src3d, r0 + t4 * P, P, npb)
                        mv, rs = stats_ln(pool, x)
                        xc = pool.tile([P, D], F32R, tag="xc")
                        nc.vector.tensor_scalar(
                            xc[:], x[:], scalar1=mv[:, 0:1], scalar2=rs[:],
                            op0=ALU.subtract, op1=ALU.mult)
                        for c in range(KC):
                            pr = psp.tile([P, P], F32, tag="tr32")
                            nc.tensor.transpose(
                                pr[:], x[:, c * P:(c + 1) * P], ident32)
                            nc.scalar.copy(
                                out=raw_blk[:, c, t4 * P:(t4 + 1) * P],
                                in_=pr[:])
                            pc = psp2.tile([P, P], F32R, tag="trr")
                            nc.tensor.transpose(
                                pc[:], xc[:, c * P:(c + 1) * P], identr)
                            nc.vector.tensor_copy(
                                ln_blk[:, c, t4 * P:(t4 + 1) * P], pc[:])
                    for c in range(KC):
                        nc.scalar.activation(
                            ln_blk[:, c, :], ln_blk[:, c, :], AF.Identity,
                            bias=gcols[bn][:, c:c + 1],
                            scale=gcols[gn][:, c:c + 1])
                    nc.gpsimd.dma_start(rawT[:, :, r0:r0 + 512], raw_blk[:])
                    nc.gpsimd.dma_start(lnT[:, :, r0:r0 + 512], ln_blk[:])
                if inner is not None:
                    inner()

        def proj_T(name, srcT, ROWS, wname, dstT, inner=None, ps_bufs=4):
            """dstT = (src @ w) transposed-out, streaming 512-row blocks."""
            with (
                tc.tile_pool(name=f"pt_{name}", bufs=2) as pool,
                tc.tile_pool(name=f"pt_w_{name}", bufs=1) as wpool,
                tc.tile_pool(name=f"pt_ps_{name}", bufs=ps_bufs,
                             space="PSUM") as ps,
            ):
                w_sb = wpool.tile([P, KC, D], F32R)
                nc.sync.dma_start(w_sb[:], w_kxn(wname))
                for blk in range(ROWS // 512):
                    r0 = blk * 512
                    src_blk = pool.tile([P, KC, 512], F32R, tag="src")
                    nc.sync.dma_start(src_blk[:], srcT[:, :, r0:r0 + 512])
                    o_blk = pool.tile([P, KC, 512], F32R, tag="o")
                    for m in range(KC):
                        acc = ps.tile([P, 512], F32, tag="acc")
                        for kc in range(KC):
                            nc.tensor.matmul(
                                acc[:], w_sb[:, kc, m * P:(m + 1) * P],
                                src_blk[:, kc, :],
                                start=(kc == 0), stop=(kc == KC - 1))
                        nc.vector.tensor_copy(o_blk[:, m, :], acc[:])
                    nc.gpsimd.dma_start(dstT[:, :, r0:r0 + 512], o_blk[:])
                if inner is not None:
                    inner()

        def proj_vpad(name, srcT, ROWS, wname, vpad):
            """v = src @ wv row-major, padded per head with a ones column."""
            with (
                tc.tile_pool(name=f"pv_{name}", bufs=2) as pool,
                tc.tile_pool(name=f"pv_w_{name}", bufs=1) as wpool,
                tc.tile_pool(name=f"pv_ps_{name}", bufs=4, space="PSUM") as ps,
            ):
                w_sb = wpool.tile([P, KC, D], F32R)
                nc.sync.dma_start(w_sb[:], w_kxn(wname))
                for blk in range(ROWS // 512):
                    r0 = blk * 512
                    src_blk = pool.tile([P, KC, 512], F32R, tag="src")
                    nc.sync.dma_start(src_blk[:], srcT[:, :, r0:r0 + 512])
                    for rc in range(4):
                        vp = pool.tile([P, H, DH + 1], F32R, tag="vp")
                        nc.vector.tensor_copy(vp[:, :, DH:DH + 1],
                                              ones16[:, :, None])
                        for nn in range(2):
                            acc = ps.tile([P, 512], F32, tag="acc")
                            for kc in range(KC):
                                nc.tensor.matmul(
                                    acc[:],
                                    src_blk[:, kc, rc * P:(rc + 1) * P],
                                    w_sb[:, kc, nn * 512:(nn + 1) * 512],
                                    start=(kc == 0), stop=(kc == KC - 1))
                            nc.vector.tensor_copy(
                                vp[:, nn * 8:(nn + 1) * 8, :DH],
                                acc.rearrange("p (h d) -> p h d", h=8))
                        nc.gpsimd.dma_start(
                            vpad[r0 + rc * P:r0 + (rc + 1) * P, :],
                            vp.rearrange("p h d -> p (h d)"))

        def attn(name, qT, nq, kT, nk, vpad, scr, scr2, dstT):
            """Cross attention: per b, per head; q rows nq, kv rows nk."""
            nkc = (nk + P - 1) // P
            pieces = [(0, nq)] if nq <= 512 else [(0, nq // 2),
                                                  (nq // 2, nq // 2)]
            with (
                tc.tile_pool(name=f"at_{name}", bufs=2) as pool,
                tc.tile_pool(name=f"at_t1_{name}", bufs=1) as tailp,
                tc.tile_pool(name=f"at_sm_{name}", bufs=2) as smp,
                tc.tile_pool(name=f"at_ex_{name}", bufs=4) as exp_pool,
                tc.tile_pool(name=f"at_ps_{name}", bufs=2, space="PSUM") as ps,
                tc.tile_pool(name=f"at_po_{name}", bufs=4,
                             space="PSUM") as pso,
            ):
                for b in range(BL):
                    kT_b = pool.tile([P, KC, nk], F32R, tag="kT")
                    nc.sync.dma_start(kT_b[:], kT[:, :, b * nk:(b + 1) * nk])
                    qT_b = pool.tile([P, KC, nq], F32R, tag="qT")
                    nc.sync.dma_start(qT_b[:], qT[:, :, b * nq:(b + 1) * nq])
                    vps = []
                    for c in range(nkc):
                        ckn = min(P, nk - c * P)
                        vt = pool.tile([P, H * (DH + 1)], F32R, tag=f"vp{c}")
                        nc.sync.dma_start(
                            vt[:ckn], vpad[b * nk + c * P:b * nk + c * P + ckn])
                        vps.append((vt, ckn))
                    oU = pool.tile([P, KC, nq], F32, tag="oU")
                    dall = tailp.tile([1, H * nq], F32, tag="dall")
                    for hp in range(H // 2):
                        h0, h1 = 2 * hp, 2 * hp + 1
                        for (qoff, qn) in pieces:
                            acc_o0 = pso.tile([DH + 1, 512], F32, tag="acco")
                            acc_o1 = pso.tile([DH + 1, 512], F32, tag="acco")
                            for c in range(nkc):
                                vt, ckn = vps[c]
                                # both heads' scores in one 2-bank psum tile;
                                # one Exp over the pair halves the ACT count
                                acc_s = ps.tile([P, 2, 512], F32, tag="accs")
                                nc.tensor.matmul(
                                    acc_s[:ckn, 0, :qn],
                                    kT_b[0:DH, hp, c * P:c * P + ckn],
                                    qT_b[0:DH, hp, qoff:qoff + qn],
                                    start=True, stop=True)
                                nc.tensor.matmul(
                                    acc_s[:ckn, 1, :qn],
                                    kT_b[DH:P, hp, c * P:c * P + ckn],
                                    qT_b[DH:P, hp, qoff:qoff + qn],
                                    start=True, stop=True)
                                et = exp_pool.tile([P, 2, 512], F32R,
                                                   tag="et")
                                nc.scalar.activation(
                                    et[:ckn, :, :qn], acc_s[:ckn, :, :qn],
                                    AF.Exp, scale=SCALE)
                                nc.tensor.matmul(
                                    acc_o0[:, :qn],
                                    vt[:ckn,
                                       h0 * (DH + 1):(h0 + 1) * (DH + 1)],
                                    et[:ckn, 0, :qn],
                                    start=(c == 0), stop=(c == nkc - 1))
                                nc.tensor.matmul(
                                    acc_o1[:, :qn],
                                    vt[:ckn,
                                       h1 * (DH + 1):(h1 + 1) * (DH + 1)],
                                    et[:ckn, 1, :qn],
                                    start=(c == 0), stop=(c == nkc - 1))
                            for h, acc_o, po in ((h0, acc_o0, 0),
                                                 (h1, acc_o1, DH)):
                                nc.vector.tensor_copy(
                                    dall[:, h * nq + qoff:
                                         h * nq + qoff + qn],
                                    acc_o[DH:DH + 1, :qn])
                                nc.vector.tensor_copy(
                                    oU[po:po + DH, hp, qoff:qoff + qn],
                                    acc_o[:DH, :qn])
                    # denominator: bounce through DRAM to reshape [1, H*nq]
                    # -> [H, nq], reciprocal, bounce back broadcast [128,KC,nq]
                    scr_b = scr[b]
                    nc.gpsimd.dma_start(
                        scr_b.rearrange("h q -> (h q)")[None, :], dall[:])
                    den16 = smp.tile([H, nq], F32, tag="den16")
                    nc.gpsimd.dma_start(den16[:], scr_b)
                    nc.vector.reciprocal(den16[:], den16[:])
                    scr2_b = scr2[b]
                    nc.gpsimd.dma_start(scr2_b, den16[:])
                    bc = tailp.tile([P, KC, nq], F32, tag="bc")
                    for h2 in range(2):
                        sl = scr2_b[h2, :]
                        nc.gpsimd.dma_start(
                            bc[h2 * 64:(h2 + 1) * 64, :, :],
                            bass.AP(tensor=sl.tensor, offset=sl.offset,
                                    ap=[[0, 64], [2 * nq, KC], [1, nq]]))
                    obn = tailp.tile([P, KC, nq], F32R, tag="obn")
                    nc.vector.tensor_mul(obn[:], oU[:], bc[:])
                    nc.gpsimd.dma_start(dstT[:, :, b * nq:(b + 1) * nq],
                                        obn[:])

        def mid_stage(name, attnT, ROWS, woname, boname, res3d, npb, gn, bn,
                      mid, xhT, bpool_bufs=2, ps_bufs=4):
            """o-proj + bias + residual -> mid; then LN -> xhat transposed."""
            with (
                tc.tile_pool(name=f"md_{name}", bufs=3) as pool,
                tc.tile_pool(name=f"md_b_{name}", bufs=bpool_bufs) as bpool,
                tc.tile_pool(name=f"md_w_{name}", bufs=1) as wpool,
                tc.tile_pool(name=f"md_ps_{name}", bufs=ps_bufs,
                             space="PSUM") as ps,
                tc.tile_pool(name=f"md_ps2_{name}", bufs=2,
                             space="PSUM") as psp2,
            ):
                w_sb = wpool.tile([P, KC, D], F32R)
                nc.sync.dma_start(w_sb[:], w_kxn(woname))
                bo_bc = wpool.tile([P, D], F32)
                nc.sync.dma_start(bo_bc[:], dram_bcast(W[boname][:], P))
                for blk in range(ROWS // 512):
                    r0 = blk * 512
                    a_blk = bpool.tile([P, KC, 512], F32R, tag="ablk")
                    nc.sync.dma_start(a_blk[:], attnT[:, :, r0:r0 + 512])
                    xh_blk = bpool.tile([P, KC, 512], F32R, tag="xhblk")
                    for rc in range(4):
                        rr = r0 + rc * P
                        t1 = pool.tile([P, D], F32, tag="t1")
                        res = pool.tile([P, D], F32, tag="res")
                        dma_rows(res, res3d, rr, P, npb)
                        for nn in range(2):
                            acc = ps.tile([P, 512], F32, tag="acc")
                            for kc in range(KC):
                                nc.tensor.matmul(
                                    acc[:], a_blk[:, kc, rc * P:(rc + 1) * P],
                                    w_sb[:, kc, nn * 512:(nn + 1) * 512],
                                    start=(kc == 0), stop=(kc == KC - 1))
                            nc.vector.tensor_add(
                                t1[:, nn * 512:(nn + 1) * 512], acc[:],
                                res[:, nn * 512:(nn + 1) * 512])
                        nc.vector.tensor_add(t1[:], t1[:], bo_bc[:])
                        nc.gpsimd.dma_start(mid[rr:rr + P, :], t1[:])
                        mv, rs = stats_ln(pool, t1)
                        xc = pool.tile([P, D], F32R, tag="xc")
                        nc.vector.tensor_scalar(
                            xc[:], t1[:], scalar1=mv[:, 0:1], scalar2=rs[:],
                            op0=ALU.subtract, op1=ALU.mult)
                        for c in range(KC):
                            pc = psp2.tile([P, P], F32R, tag="trr")
                            nc.tensor.transpose(
                                pc[:], xc[:, c * P:(c + 1) * P], identr)
                            nc.scalar.copy(
                                out=xh_blk[:, c, rc * P:(rc + 1) * P],
                                in_=pc[:])
                    for c in range(KC):
                        nc.scalar.activation(
                            xh_blk[:, c, :], xh_blk[:, c, :], AF.Identity,
                            bias=gcols[bn][:, c:c + 1],
                            scale=gcols[gn][:, c:c + 1])
                    nc.gpsimd.dma_start(xhT[:, :, r0:r0 + 512], xh_blk[:])

        def ffn_stage(name, xhT, ROWS, w1name, b1name, w2name, b2name,
                      mid, y0, out3d, npb):
            """h = gelu(xh @ w1 + b1); y = h @ w2 + b2 + mid.

            HID processed in quarters of 1024 with a ping-pong weight pool
            (bufs=2): quarter q+1 weights DMA during quarter q compute."""
            QKC = 8                    # 1024/128 k-chunks per quarter
            with (
                tc.tile_pool(name=f"ff_{name}", bufs=2) as pool,
                tc.tile_pool(name=f"ff_b_{name}", bufs=1) as bpool,
                tc.tile_pool(name=f"ff_w_{name}", bufs=2) as wpool,
                tc.tile_pool(name=f"ff_ps_{name}", bufs=4,
                             space="PSUM") as ps,
                tc.tile_pool(name=f"ff_ps2_{name}", bufs=4,
                             space="PSUM") as ps2,
            ):
                b2_bc = bpool.tile([P, D], F32)
                nc.sync.dma_start(b2_bc[:], dram_bcast(W[b2name][:], P))
                for qt in range(4):
                    w1_sb = wpool.tile([P, KC, 1024], F32R, tag="w1")
                    nc.sync.dma_start(
                        w1_sb[:],
                        w_kxn(w1name)[:, :, qt * 1024:(qt + 1) * 1024])
                    w2_sb = wpool.tile([P, QKC, D], F32R, tag="w2")
                    nc.sync.dma_start(
                        w2_sb[:], w_kxn(w2name)[:, qt * QKC:(qt + 1) * QKC, :])
                    for blk in range(ROWS // 256):
                        r0 = blk * 256
                        xh_blk = pool.tile([P, KC, 256], F32R, tag="xh")
                        nc.sync.dma_start(xh_blk[:], xhT[:, :, r0:r0 + 256])
                        hT_blk = pool.tile([P, QKC, 256], F32R, tag="hT")
                        for m in range(QKC):
                            acc = ps.tile([P, 256], F32, tag="acc")
                            for kc in range(KC):
                                nc.tensor.matmul(
                                    acc[:], w1_sb[:, kc, m * P:(m + 1) * P],
                                    xh_blk[:, kc, :],
                                    start=(kc == 0), stop=(kc == KC - 1))
                            nc.scalar.activation(
                                hT_blk[:, m, :], acc[:], AF.Gelu,
                                bias=b1col[b1name][:, qt * QKC + m:
                                                   qt * QKC + m + 1])
                        for rc in range(2):
                            rr = r0 + rc * P
                            yt = pool.tile([P, D], F32, tag="yt")
                            for nn in range(2):
                                acc2 = ps2.tile([P, 512], F32, tag="acc2")
                                for kc in range(QKC):
                                    nc.tensor.matmul(
                                        acc2[:],
                                        hT_blk[:, kc, rc * P:(rc + 1) * P],
                                        w2_sb[:, kc, nn * 512:(nn + 1) * 512],
                                        start=(kc == 0), stop=(kc == QKC - 1))
                                if qt == 0:
                                    nc.vector.tensor_add(
                                        yt[:, nn * 512:(nn + 1) * 512],
                                        acc2[:],
                                        b2_bc[:, nn * 512:(nn + 1) * 512])
                                else:
                                    y0t = pool.tile([P, 512], F32, tag="y0t")
                                    nc.sync.dma_start(
                                        y0t[:],
                                        y0[rr:rr + P,
                                           nn * 512:(nn + 1) * 512])
                                    nc.vector.tensor_add(
                                        yt[:, nn * 512:(nn + 1) * 512],
                                        acc2[:], y0t[:])
                            if qt < 3:
                                nc.gpsimd.dma_start(y0[rr:rr + P, :], yt[:])
                            else:
                                mt = pool.tile([P, D], F32, tag="mt")
                                nc.sync.dma_start(mt[:], mid[rr:rr + P, :])
                                nc.vector.tensor_add(yt[:], yt[:], mt[:])
                                dma_rows(yt, out3d, rr, P, npb,
                                         store=True)

        # interleave the two independent branches: every stage's producer
        # is >=2 stages back, so adjacent stages pipeline on the engines
        with nc.named_scope("prep_t"):
            prep_stage("t", tok_bnd, NT, RT, "tn_g", "tn_b", TtT, LtT)
        def _t_projs():
            # PE-dense projections depending only on prep_t outputs; run
            # inside prep_i's pool scope so the stack allocator places them
            # above prep_i's live pools -> no reuse dep -> true overlap of
            # prep_i's DVE work with PE matmuls.
            proj_T("qt", LtT, RT, "t2i_wq", qtT)
            proj_T("kt", TtT, RT, "i2t_wk", ktT)
            proj_vpad("vt", TtT, RT, "i2t_wv", vpad_t)

        with nc.named_scope("prep_i"):
            prep_stage("i", img_bnd, NI, RI, "in_g", "in_b", TiT, LiT,
                       inner=_t_projs)
        with nc.named_scope("proj_ki"):
            proj_T("ki", TiT, RI, "t2i_wk", kiT)
        with nc.named_scope("proj_vi"):
            proj_vpad("vi", TiT, RI, "t2i_wv", vpad_i)
        with nc.named_scope("attn_t2i"):
            attn("t2i", qtT, NT, kiT, NI, vpad_i, scr_t, scr2_t, attnT_t)
        def _mid_t():
            # mid_t depends only on attn_t2i; nested inside proj_qi's pool
            # scope so its DVE/ACT chains overlap proj_qi's PE matmuls
            mid_stage("t", attnT_t, RT, "t2i_wo", "t2i_bo", tok_bnd, NT,
                      "tfn_g", "tfn_b", tmid, xhT_t, bpool_bufs=1,
                      ps_bufs=2)

        with nc.named_scope("proj_qi"):
            proj_T("qi", LiT, RI, "i2t_wq", qiT, inner=_mid_t, ps_bufs=2)
        with nc.named_scope("attn_i2t"):
            attn("i2t", qiT, NI, ktT, NT, vpad_t, scr_i, scr2_i, attnT_i)
        with nc.named_scope("mid_i"):
            mid_stage("i", attnT_i, RI, "i2t_wo", "i2t_bo", img_bnd, NI,
                      "ifn_g", "ifn_b", imid, xhT_i)
        with nc.named_scope("ffn_t"):
            ffn_stage("t", xhT_t, RT, "tf_w1", "tf_b1", "tf_w2", "tf_b2",
                      tmid, y0_t, tout_bnd, NT)
        with nc.named_scope("ffn_i"):
            ffn_stage("i", xhT_i, RI, "if_w1", "if_b1", "if_w2", "if_b2",
                      imid, y0_i, iout_bnd, NI)

        singles_cm.__exit__(None, None, None)
        dram_cm.__exit__(None, None, None)

    nc.compile()
    return nc


def kernel(**inputs):
    if "nc" not in _CACHE:
        _CACHE["nc"] = build()
    nc = _CACHE["nc"]
    in_maps = []
    for core in range(NCORES):
        b0 = core * BL
        m = {"tokens": np.ascontiguousarray(inputs["tokens"][:, b0:b0 + BL]),
             "image_features": np.ascontiguousarray(
                 inputs["image_features"][:, b0:b0 + BL])}
        for n in WNAMES:
            m[n] = np.asarray(inputs[n], dtype=np.float32)
        in_maps.append(m)
    res = run_bass_kernel_spmd(nc, in_maps, list(range(NCORES)))
    t = np.concatenate([r["t_out"] for r in res.results], axis=1)
    im = np.concatenate([r["im_out"] for r in res.results], axis=1)
    return (t, im)


if __name__ == "__main__":
    rng = np.random.default_rng(0)
    ins = {
        "tokens": rng.standard_normal((NT, BL * NCORES, D)).astype(np.float32),
        "image_features": rng.standard_normal((NI, BL * NCORES, D)).astype(
            np.float32),
    }
    for n in WNAMES:
        sh = WSHAPES[n]
        if n.endswith("_g"):
            ins[n] = np.ones(sh, np.float32)
        elif n.endswith(("_b", "_bo", "b1", "b2")):
            ins[n] = np.zeros(sh, np.float32)
        else:
            ins[n] = (rng.standard_normal(sh) * 0.02).astype(np.float32)
    t, im = kernel(**ins)
    print("t", t.shape, "im", im.shape)
